# revision 8
# baseline (speedup 1.0000x reference)
"""Multi-head causal attention (B=2, S=2048, D=1024, H=16) on 8 NeuronCores.

Sharding: core c = (batch b=c//4, head-group g=c%4 of 4 heads).
Schedule per core:
  1. qk projection for both head pairs (d-outer over 8 PSUM banks so matmuls
     fire as the per-block weight/xT DMAs land), then V projection.
  2. Attention pair-by-pair with BOTH heads of a pair packed per key-block:
     the two K=64 score matmuls sit in PE row-groups 0-1 / 2-3 (tile_position
     auto-derived from base partitions 0/64) and run concurrently; softmax
     exp is split across engines (head A: Schraudolph int16-bitcast exp on
     the vector engine, head B: ACT-table exp on the scalar engine); the A@V
     matmuls accumulate the fused ones-column denominator (VW=65).
     AV for key-block kb is emitted after the scores of kb+1 (lag-1 software
     pipeline) so the PE never waits on the exp engines.
  3. Two 8-core AllToAlls swap head-shards for query-shards (one per pair;
     pair-0's fires at the attention midpoint). A tiny warmup AllToAll at
     kernel start absorbs the first-collective setup cost.
  4. Output projection on a fixed local 256-query slice of each batch:
     pair-0 f-blocks accumulate during the second AllToAll's flight, pair-1
     blocks after it lands; outputs drain per-tile across four DMA queues.

PSUM budget (8 banks): tags a/b/c/d, each 2 bufs of [128,512]f32 slots.
"""

import numpy as np

import concourse.bass as bass
import concourse.mybir as mybir
import concourse.tile as tile
from concourse import bacc
from concourse.bass_utils import run_bass_kernel_spmd

B, S, D = 2, 2048, 1024
H = 16
DH = 64  # head dim
N_CORES = 8
GROUPS = 4  # cores per batch = head groups
H_LOC = H // GROUPS  # 4 heads per core
EH = H_LOC * DH  # 256 local qkv width
QCH = 512  # query chunk
KB = 128  # key block
NKB = S // KB  # 16
NDB = D // 128  # 8 contraction blocks
QL = 256  # local output query rows per batch
VW = DH + 1  # 65: V columns + fused ones column
SCALE = 1.0 / 8.0  # 1/sqrt(DH)

# Schraudolph exp on the DVE: i16 = round(score*SCALE*128/ln2 + (127*128-c));
# the int16 bit pattern read as bf16 approximates exp(score*SCALE) (+-3%).
A_EXP = SCALE * 184.6650230929499  # 128/ln(2) * SCALE
B_EXP = 16248.6  # 127*128 - 7.4 (mid-point bias correction)

F32 = mybir.dt.float32
BF16 = mybir.dt.bfloat16
I16 = mybir.dt.int16
MM_DT = BF16
EXP = mybir.ActivationFunctionType.Exp
MULT = mybir.AluOpType.mult
ADD = mybir.AluOpType.add


def _emit(nc, tc, xT, wq_d, wk_d, wv_d, wo_d, bb_d, y_d):
    from contextlib import ExitStack

    ctx = ExitStack()
    with ctx:
        persist = ctx.enter_context(tc.tile_pool(name="persist", bufs=1))
        psum = ctx.enter_context(tc.tile_pool(name="psum", bufs=1, space="PSUM"))
        dram = ctx.enter_context(tc.tile_pool(name="dram", bufs=1, space="DRAM"))
        work = ctx.enter_context(tc.tile_pool(name="work", bufs=1))

        PTAGS = ["a", "b", "c", "d"]

        # --- constants ---
        ones_f = persist.tile([128, 1], F32)
        nc.vector.memset(ones_f[:], 1.0)
        dmy = persist.tile([128, 512], MM_DT)
        nc.vector.memset(dmy[:], 0.0)
        # tri[k, t] = 1 if t >= k else 0 (bf16): causal mask for a diagonal
        # 128-key x 128-query sub-block. (gpsimd: affine_select lives there.)
        tri = persist.tile([128, 128], MM_DT)
        nc.gpsimd.memset(tri[:], 1.0)
        nc.gpsimd.affine_select(
            out=tri[:],
            in_=tri[:],
            compare_op=mybir.AluOpType.is_ge,
            fill=0.0,
            base=0,
            channel_multiplier=-1,
            pattern=[[1, 128]],
        )
        bb_sb = persist.tile([128, D], F32)

        # --- persistent operand tiles ---
        xt_sb = [persist.tile([128, S], MM_DT, name=f"xt{d}") for d in range(NDB)]
        w_sb = {
            nm: persist.tile([128, NDB * EH], MM_DT, name=f"w{nm}sb")
            for nm in ("q", "k", "v")
        }
        wo_sb = persist.tile([128, NDB * D], MM_DT)
        qt = [persist.tile([128, S], MM_DT, name=f"qt{p}") for p in range(2)]
        kt = [persist.tile([128, S], MM_DT, name=f"kt{p}") for p in range(2)]
        vg = [persist.tile([128, NKB * VW], MM_DT, name=f"vg{h}") for h in range(H_LOC)]
        for h in range(H_LOC):
            nc.vector.tensor_copy(
                vg[h].rearrange("p (n w) -> p n w", w=VW)[:, :, DH : DH + 1],
                ones_f[:].unsqueeze(2).broadcast_to([128, NKB, 1]),
            )
        oft_own = [persist.tile([128, S], MM_DT, name=f"oftown{p}") for p in range(2)]
        oft_all = [persist.tile([128, 2 * QL], MM_DT, name=f"oft{f}") for f in range(NDB)]

        # --- input DMAs: per-d-block, round-robin over the DMA-capable
        # queues (sync/scalar) so the d-outer projection only waits for the
        # blocks it needs next; wv on gpsimd (needed later, at V-proj) ---
        qs = [nc.sync, nc.scalar]
        for d in range(NDB):
            q = qs[d % 2]
            q.dma_start(w_sb["q"][:, d * EH : (d + 1) * EH], wq_d[d * 128 : (d + 1) * 128, :])
            q.dma_start(w_sb["k"][:, d * EH : (d + 1) * EH], wk_d[d * 128 : (d + 1) * 128, :])
            q.dma_start(xt_sb[d][:], xT[d * 128 : (d + 1) * 128, :])
        for d in range(NDB):
            nc.gpsimd.dma_start(
                w_sb["v"][:, d * EH : (d + 1) * EH], wv_d[d * 128 : (d + 1) * 128, :]
            )

        # --- warmup AllToAll: absorbs the first-collective setup cost on the
        # CC core so the real pair-0 AllToAll starts promptly ---
        zw = persist.tile([8, 16], MM_DT)
        nc.vector.memset(zw[:], 0.0)
        zwd = dram.tile([8, 16], MM_DT, name="zwd")
        zwo = dram.tile([8, 16], MM_DT, name="zwo")
        nc.sync.dma_start(zwd[:], zw[:])
        nc.gpsimd.collective_compute(
            "AllToAll",
            mybir.AluOpType.bypass,
            replica_groups=[list(range(N_CORES))],
            ins=[zwd[:]],
            outs=[zwo[:]],
        )

        _dmy_n = [0]

        def warm_burst(n):
            # keep the PE HAM clock-gate at full rate while real matmuls are
            # DMA-gated; results are discarded
            for _ in range(n):
                i = _dmy_n[0]
                _dmy_n[0] += 1
                ps = psum.tile(
                    [128, 512], F32, tag=PTAGS[i % 2], bufs=2, name=f"dmy{i}"
                )
                nc.tensor.matmul(ps[:], dmy[:, 0:128], dmy[:], start=True, stop=True)

        # --- qk projection for one pair: d-outer over all 8 (q/k, chunk)
        # PSUM tiles so matmuls fire as each (w, xT) d-block arrives.
        # PSUM->SBUF copies go to the scalar engine (idle during proj). ---
        def emit_qk_phase(pair):
            tiles = {}
            order = [(nm, jc) for nm in ("q", "k") for jc in range(4)]
            for i, (nm, jc) in enumerate(order):
                tiles[(nm, jc)] = psum.tile(
                    [128, QCH], F32, tag=PTAGS[i % 4], bufs=2, name=f"pp{pair}{nm}{jc}"
                )
            for d in range(NDB):
                for (nm, jc), ps in tiles.items():
                    nc.tensor.matmul(
                        ps[:],
                        w_sb[nm][:, d * EH + 128 * pair : d * EH + 128 * pair + 128],
                        xt_sb[d][:, jc * QCH : (jc + 1) * QCH],
                        start=(d == 0),
                        stop=(d == NDB - 1),
                    )
            for (nm, jc), ps in tiles.items():
                dst = (qt if nm == "q" else kt)[pair]
                nc.scalar.copy(dst[:, jc * QCH : (jc + 1) * QCH], ps[:])

        # --- V projection for one 128-key block (copies on the DVE) ---
        def emit_proj_v(sb_i):
            ps = psum.tile(
                [128, EH], F32, tag=PTAGS[sb_i % 4], bufs=2, name=f"pv{sb_i}"
            )
            for d in range(NDB):
                nc.tensor.matmul(
                    ps[:],
                    xt_sb[d][:, sb_i * KB : (sb_i + 1) * KB],
                    w_sb["v"][:, d * EH : (d + 1) * EH],
                    start=(d == 0),
                    stop=(d == NDB - 1),
                )
            for h in range(H_LOC):
                nc.vector.tensor_copy(
                    vg[h][:, sb_i * VW : sb_i * VW + DH],
                    ps[:, h * DH : (h + 1) * DH],
                )

        # --- wo blocks + bias (gpsimd queue, spread across attention) ---
        def emit_wo_block(f):
            nc.gpsimd.dma_start(
                wo_sb[:, f * D : (f + 1) * D], wo_d[f * 128 : (f + 1) * 128, :]
            )

        # --- attention for pair p, query chunk j: both heads packed.
        # Score matmuls of the two heads are adjacent -> concurrent PE
        # row-groups. AV lags the scores by one key-block so the PE never
        # stalls on the exp engines. ---
        def emit_av(p, j, nkb, kb, c0, eA, eB, potA, potB):
            nc.tensor.matmul(
                potA[:, c0:QCH],
                vg[2 * p][:, kb * VW : (kb + 1) * VW],
                eA[:, c0:QCH],
                start=(kb == 0),
                stop=(kb == nkb - 1),
            )
            nc.tensor.matmul(
                potB[:, c0:QCH],
                vg[2 * p + 1][:, kb * VW : (kb + 1) * VW],
                eB[:, c0:QCH],
                start=(kb == 0),
                stop=(kb == nkb - 1),
            )

        def emit_attn_pair_chunk(p, j):
            nkb = 4 * (j + 1)
            potA = psum.tile([VW, QCH], F32, tag="c", bufs=2, name=f"potA{p}_{j}")
            potB = psum.tile([VW, QCH], F32, tag="d", bufs=2, name=f"potB{p}_{j}")
            pend = None
            for kb in range(nkb):
                c0 = max(0, KB * kb - QCH * j)
                pssA = psum.tile([128, QCH], F32, tag="a", bufs=2, name=f"psA{p}{j}{kb}")
                pssB = psum.tile([128, QCH], F32, tag="b", bufs=2, name=f"psB{p}{j}{kb}")
                nc.tensor.matmul(
                    pssA[:, c0:QCH],
                    kt[p][0:DH, kb * KB : (kb + 1) * KB],
                    qt[p][0:DH, j * QCH + c0 : (j + 1) * QCH],
                    start=True,
                    stop=True,
                )
                nc.tensor.matmul(
                    pssB[:, c0:QCH],
                    kt[p][DH : 2 * DH, kb * KB : (kb + 1) * KB],
                    qt[p][DH : 2 * DH, j * QCH + c0 : (j + 1) * QCH],
                    start=True,
                    stop=True,
                )
                if pend is not None:
                    emit_av(p, j, nkb, *pend, potA, potB)
                eA = work.tile([128, QCH], MM_DT, tag="eA", bufs=3, name=f"eA{p}{j}{kb}")
                eB = work.tile([128, QCH], MM_DT, tag="eB", bufs=3, name=f"eB{p}{j}{kb}")
                # head A: Schraudolph exp on DVE; head B: table exp on ACT
                if True:  # bisect: ACT exp for head A as well
                    nc.scalar.activation(
                        eA[:, c0:QCH], pssA[:, c0:QCH], EXP, scale=SCALE
                    )
                else:
                    nc.vector.tensor_scalar(
                        eA[:, c0:QCH].bitcast(I16),
                        pssA[:, c0:QCH],
                        A_EXP,
                        B_EXP,
                        op0=MULT,
                        op1=ADD,
                    )
                nc.scalar.activation(
                    eB[:, c0:QCH], pssB[:, c0:QCH], EXP, scale=SCALE
                )
                m = kb - 4 * j
                if 0 <= m <= 3:  # diagonal sub-block: zero the upper triangle
                    ct = 128 * m
                    for e_t in (eA, eB):
                        nc.vector.tensor_tensor(
                            e_t[:, ct : ct + 128], e_t[:, ct : ct + 128], tri[:], op=MULT
                        )
                pend = (kb, c0, eA, eB)
            emit_av(p, j, nkb, *pend, potA, potB)
            # normalize both heads: oft_own = pot[0:64] / pot[64].
            # (den staged through SBUF: custom-DVE reciprocal can't read PSUM)
            den = work.tile([1, 2 * QCH], F32, tag="den", bufs=2, name=f"den{p}_{j}")
            nc.vector.tensor_copy(den[:, 0:QCH], potA[DH : DH + 1, :])
            nc.vector.tensor_copy(den[:, QCH : 2 * QCH], potB[DH : DH + 1, :])
            rec = work.tile([1, 2 * QCH], F32, tag="rec", bufs=2, name=f"rec{p}_{j}")
            nc.vector.reciprocal_approx_fast(rec[:], den[:])
            pb = work.tile([DH, 2 * QCH], F32, tag="pb", bufs=2, name=f"pb{p}_{j}")
            nc.gpsimd.partition_broadcast(pb[:], rec[0:1, :])
            nc.vector.tensor_tensor(
                oft_own[p][0:DH, j * QCH : (j + 1) * QCH],
                potA[0:DH, :],
                pb[:, 0:QCH],
                op=MULT,
            )
            nc.vector.tensor_tensor(
                oft_own[p][DH : 2 * DH, j * QCH : (j + 1) * QCH],
                potB[0:DH, :],
                pb[:, QCH : 2 * QCH],
                op=MULT,
            )

        # --- A2A plumbing ---
        a2a_bufs = {}

        def emit_a2a_cin(p, j):
            if p not in a2a_bufs:
                cin = dram.tile([N_CORES * 128, QL], MM_DT, name=f"cin{p}")
                cout = dram.tile([N_CORES * 128, QL], MM_DT, name=f"cout{p}")
                a2a_bufs[p] = (cin, cout)
            cin = a2a_bufs[p][0]
            for s in (2 * j, 2 * j + 1):
                nc.scalar.dma_start(
                    cin[s * 128 : (s + 1) * 128, :],
                    oft_own[p][:, s * QL : (s + 1) * QL],
                )

        def emit_a2a_trigger(p):
            cin, cout = a2a_bufs[p]
            nc.gpsimd.collective_compute(
                "AllToAll",
                mybir.AluOpType.bypass,
                replica_groups=[list(range(N_CORES))],
                ins=[cin[:]],
                outs=[cout[:]],
            )

        def emit_a2a_post(p):
            cin, cout = a2a_bufs[p]
            for rr in range(GROUPS):
                for bi in range(2):
                    src_rank = bi * GROUPS + rr
                    nc.sync.dma_start(
                        oft_all[2 * rr + p][:, bi * QL : (bi + 1) * QL],
                        cout[src_rank * 128 : (src_rank + 1) * 128, :],
                    )

        # ===== emission schedule =====
        warm_burst(10)
        emit_qk_phase(0)
        emit_qk_phase(1)
        for sb_i in range(NKB):
            emit_proj_v(sb_i)

        emit_attn_pair_chunk(0, 0)
        emit_a2a_cin(0, 0)
        emit_wo_block(0)
        emit_attn_pair_chunk(0, 1)
        emit_a2a_cin(0, 1)
        emit_wo_block(1)
        emit_attn_pair_chunk(0, 2)
        emit_a2a_cin(0, 2)
        emit_wo_block(2)
        emit_attn_pair_chunk(0, 3)
        emit_a2a_cin(0, 3)
        emit_a2a_trigger(0)
        emit_wo_block(3)

        emit_attn_pair_chunk(1, 3)
        emit_a2a_cin(1, 3)
        emit_wo_block(4)
        emit_wo_block(5)
        emit_attn_pair_chunk(1, 2)
        emit_a2a_cin(1, 2)
        emit_wo_block(6)
        emit_wo_block(7)
        emit_attn_pair_chunk(1, 1)
        emit_a2a_cin(1, 1)
        nc.gpsimd.dma_start(bb_sb[:], bb_d[:])
        emit_attn_pair_chunk(1, 0)
        emit_a2a_cin(1, 0)
        emit_a2a_post(0)
        emit_a2a_trigger(1)
        emit_a2a_post(1)

        # --- output projection on local 256-query slice of each batch ---
        # 8 PSUM slots (4 tiles x 2 column halves) held through both phases.
        out_tiles = [(0, 0), (0, 1), (1, 0), (1, 1)]  # (bi, qb)
        pys = [
            psum.tile([128, QCH], F32, tag=PTAGS[t], bufs=2, name=f"py{t}_{ech}")
            for t in range(4)
            for ech in range(2)
        ]

        # Gate matmuls: zero contribution (moving operand is the zeros tile),
        # but the stationary operand reads oft_own[1] cols 0:128 — written by
        # the LAST attention normalize (pair 1, chunk 0). Deliberate fence:
        # the PE queue is in-order, so without it the scheduler hoists these
        # out-proj accumulations (which wait on the collective's DMAs) into
        # the attention stream and stalls attention behind the A2A.
        for i in range(8):
            nc.tensor.matmul(
                pys[i][:], oft_own[1][:, 0:128], dmy[:], start=True, stop=False
            )

        def emit_out_phase(fs, last):
            for t, (bi, qb) in enumerate(out_tiles):
                for fi, f in enumerate(fs):
                    for ech in range(2):
                        nc.tensor.matmul(
                            pys[2 * t + ech][:],
                            oft_all[f][:, bi * QL + qb * 128 : bi * QL + (qb + 1) * 128],
                            wo_sb[:, f * D + ech * QCH : f * D + (ech + 1) * QCH],
                            start=False,
                            stop=(last and fi == len(fs) - 1),
                        )
                if last:  # per-tile drain: bias add + y DMA on rotating queues
                    ysb = work.tile([128, D], F32, tag="ysb", bufs=2, name=f"y{t}")
                    for ech in range(2):
                        nc.vector.tensor_tensor(
                            ysb[:, ech * QCH : (ech + 1) * QCH],
                            pys[2 * t + ech][:],
                            bb_sb[:, ech * QCH : (ech + 1) * QCH],
                            op=ADD,
                        )
                    row0 = bi * QL + qb * 128
                    for half in range(2):
                        qs[half % 2].dma_start(
                            y_d[row0 : row0 + 128, half * QCH : (half + 1) * QCH],
                            ysb[:, half * QCH : (half + 1) * QCH],
                        )

        emit_out_phase([0, 2, 4, 6], last=False)
        emit_out_phase([1, 3, 5, 7], last=True)


def build_program():
    nc = bacc.Bacc(
        "TRN2", target_bir_lowering=False, debug=False, num_devices=N_CORES
    )
    xT = nc.dram_tensor("xT", [D, S], BF16, kind="ExternalInput")
    wq = nc.dram_tensor("wq", [D, EH], BF16, kind="ExternalInput")
    wk = nc.dram_tensor("wk", [D, EH], BF16, kind="ExternalInput")
    wv = nc.dram_tensor("wv", [D, EH], BF16, kind="ExternalInput")
    wo = nc.dram_tensor("wo", [D, D], BF16, kind="ExternalInput")
    bb = nc.dram_tensor("bb", [128, D], F32, kind="ExternalInput")
    y = nc.dram_tensor("y", [2 * QL, D], F32, kind="ExternalOutput")
    with tile.TileContext(nc) as tc:
        _emit(nc, tc, xT.ap(), wq.ap(), wk.ap(), wv.ap(), wo.ap(), bb.ap(), y.ap())
    nc.compile()
    return nc


_cached_nc = None


def _get_nc():
    global _cached_nc
    if _cached_nc is None:
        _cached_nc = build_program()
    return _cached_nc


def make_in_maps(x, w_qkv, w_out, b_out):
    import ml_dtypes

    bf16 = ml_dtypes.bfloat16
    x = np.asarray(x, np.float32).astype(bf16)
    w_qkv = np.asarray(w_qkv, np.float32).astype(bf16)
    w_out = np.ascontiguousarray(np.asarray(w_out, np.float32).astype(bf16))
    b_out = np.asarray(b_out, np.float32)
    bb = np.ascontiguousarray(np.broadcast_to(b_out, (128, D)))
    in_maps = []
    for c in range(N_CORES):
        b, g = c // GROUPS, c % GROUPS
        in_maps.append(
            {
                "xT": np.ascontiguousarray(x[b].T),
                "wq": np.ascontiguousarray(w_qkv[:, g * EH : (g + 1) * EH]),
                "wk": np.ascontiguousarray(w_qkv[:, D + g * EH : D + (g + 1) * EH]),
                "wv": np.ascontiguousarray(
                    w_qkv[:, 2 * D + g * EH : 2 * D + (g + 1) * EH]
                ),
                "wo": w_out,
                "bb": bb,
            }
        )
    return in_maps


def assemble(results):
    # core c's y is [512, D]: rows [0,256) = batch 0 q-slice [256c, 256c+256),
    # rows [256,512) = batch 1 same slice.
    y = np.empty((B, S, D), np.float32)
    for c in range(N_CORES):
        yc = results[c]["y"]
        y[0, 256 * c : 256 * (c + 1), :] = yc[:256]
        y[1, 256 * c : 256 * (c + 1), :] = yc[256:]
    return y


def kernel(x, w_qkv, w_out, b_out, _trace=False, **run_kwargs):
    nc = _get_nc()
    in_maps = make_in_maps(x, w_qkv, w_out, b_out)
    res = run_bass_kernel_spmd(
        nc, in_maps, core_ids=list(range(N_CORES)), trace=_trace, **run_kwargs
    )
    out = assemble(res.results)
    if _trace:
        return out, res
    return out


# revision 9
# speedup vs baseline: 1.0275x; 1.0275x over previous
"""Multi-head causal attention (B=2, S=2048, D=1024, H=16) on 8 NeuronCores.

Sharding: core c = (batch b=c//4, head-group g=c%4 of 4 heads).
Schedule per core:
  1. qk projection for both head pairs (d-outer over 8 PSUM banks so matmuls
     fire as the per-block weight/xT DMAs land), then V projection.
  2. Attention pair-by-pair with BOTH heads of a pair packed per key-block:
     the two K=64 score matmuls sit in PE row-groups 0-1 / 2-3 (tile_position
     auto-derived from base partitions 0/64) and run concurrently; softmax
     exp is split across engines (head A: Schraudolph int16-bitcast exp on
     the vector engine, head B: ACT-table exp on the scalar engine); the A@V
     matmuls accumulate the fused ones-column denominator (VW=65).
     AV for key-block kb is emitted after the scores of kb+1 (lag-1 software
     pipeline) so the PE never waits on the exp engines.
  3. Two 8-core AllToAlls swap head-shards for query-shards (one per pair;
     pair-0's fires at the attention midpoint). A tiny warmup AllToAll at
     kernel start absorbs the first-collective setup cost.
  4. Output projection on a fixed local 256-query slice of each batch:
     pair-0 f-blocks accumulate during the second AllToAll's flight, pair-1
     blocks after it lands; outputs drain per-tile across four DMA queues.

PSUM budget (8 banks): tags a/b/c/d, each 2 bufs of [128,512]f32 slots.
"""

import numpy as np

import concourse.bass as bass
import concourse.mybir as mybir
import concourse.tile as tile
from concourse import bacc
from concourse.bass_utils import run_bass_kernel_spmd

B, S, D = 2, 2048, 1024
H = 16
DH = 64  # head dim
N_CORES = 8
GROUPS = 4  # cores per batch = head groups
H_LOC = H // GROUPS  # 4 heads per core
EH = H_LOC * DH  # 256 local qkv width
QCH = 512  # query chunk
KB = 128  # key block
NKB = S // KB  # 16
NDB = D // 128  # 8 contraction blocks
QL = 256  # local output query rows per batch
VW = DH + 1  # 65: V columns + fused ones column
SCALE = 1.0 / 8.0  # 1/sqrt(DH)

# Schraudolph exp on the DVE: i16 = round(score*SCALE*128/ln2 + (127*128-c));
# the int16 bit pattern read as bf16 approximates exp(score*SCALE) (+-3%).
A_EXP = SCALE * 184.6650230929499  # 128/ln(2) * SCALE
B_EXP = 16248.6  # 127*128 - 7.4 (mid-point bias correction)

F32 = mybir.dt.float32
BF16 = mybir.dt.bfloat16
I16 = mybir.dt.int16
MM_DT = BF16
EXP = mybir.ActivationFunctionType.Exp
MULT = mybir.AluOpType.mult
ADD = mybir.AluOpType.add


def _emit(nc, tc, xT, wq_d, wk_d, wv_d, wo_d, bb_d, y_d):
    from contextlib import ExitStack

    ctx = ExitStack()
    with ctx:
        persist = ctx.enter_context(tc.tile_pool(name="persist", bufs=1))
        psum = ctx.enter_context(tc.tile_pool(name="psum", bufs=1, space="PSUM"))
        dram = ctx.enter_context(tc.tile_pool(name="dram", bufs=1, space="DRAM"))
        work = ctx.enter_context(tc.tile_pool(name="work", bufs=1))

        PTAGS = ["a", "b", "c", "d"]

        # --- constants ---
        ones_f = persist.tile([128, 1], F32)
        nc.vector.memset(ones_f[:], 1.0)
        dmy = persist.tile([128, 512], MM_DT)
        nc.vector.memset(dmy[:], 0.0)
        # tri[k, t] = 1 if t >= k else 0 (bf16): causal mask for a diagonal
        # 128-key x 128-query sub-block. (gpsimd: affine_select lives there.)
        tri = persist.tile([128, 128], MM_DT)
        nc.gpsimd.memset(tri[:], 1.0)
        nc.gpsimd.affine_select(
            out=tri[:],
            in_=tri[:],
            compare_op=mybir.AluOpType.is_ge,
            fill=0.0,
            base=0,
            channel_multiplier=-1,
            pattern=[[1, 128]],
        )
        bb_sb = persist.tile([128, D], F32)

        # --- persistent operand tiles ---
        xt_sb = [persist.tile([128, S], MM_DT, name=f"xt{d}") for d in range(NDB)]
        w_sb = {
            nm: persist.tile([128, NDB * EH], MM_DT, name=f"w{nm}sb")
            for nm in ("q", "k", "v")
        }
        wo_sb = persist.tile([128, NDB * D], MM_DT)
        qt = [persist.tile([128, S], MM_DT, name=f"qt{p}") for p in range(2)]
        kt = [persist.tile([128, S], MM_DT, name=f"kt{p}") for p in range(2)]
        vg = [persist.tile([128, NKB * VW], MM_DT, name=f"vg{h}") for h in range(H_LOC)]
        for h in range(H_LOC):
            nc.vector.tensor_copy(
                vg[h].rearrange("p (n w) -> p n w", w=VW)[:, :, DH : DH + 1],
                ones_f[:].unsqueeze(2).broadcast_to([128, NKB, 1]),
            )
        oft_own = [persist.tile([128, S], MM_DT, name=f"oftown{p}") for p in range(2)]
        oft_all = [persist.tile([128, 2 * QL], MM_DT, name=f"oft{f}") for f in range(NDB)]

        # --- input DMAs: per-d-block, round-robin over the DMA-capable
        # queues (sync/scalar) so the d-outer projection only waits for the
        # blocks it needs next; wv on gpsimd (needed later, at V-proj) ---
        qs = [nc.sync, nc.scalar]
        for d in range(NDB):
            q = qs[d % 2]
            q.dma_start(w_sb["q"][:, d * EH : (d + 1) * EH], wq_d[d * 128 : (d + 1) * 128, :])
            q.dma_start(w_sb["k"][:, d * EH : (d + 1) * EH], wk_d[d * 128 : (d + 1) * 128, :])
            q.dma_start(xt_sb[d][:], xT[d * 128 : (d + 1) * 128, :])
        for d in range(NDB):
            nc.gpsimd.dma_start(
                w_sb["v"][:, d * EH : (d + 1) * EH], wv_d[d * 128 : (d + 1) * 128, :]
            )

        # --- warmup AllToAll: absorbs the first-collective setup cost on the
        # CC core so the real pair-0 AllToAll starts promptly ---
        zw = persist.tile([8, 16], MM_DT)
        nc.vector.memset(zw[:], 0.0)
        zwd = dram.tile([8, 16], MM_DT, name="zwd")
        zwo = dram.tile([8, 16], MM_DT, name="zwo")
        nc.sync.dma_start(zwd[:], zw[:])
        nc.gpsimd.collective_compute(
            "AllToAll",
            mybir.AluOpType.bypass,
            replica_groups=[list(range(N_CORES))],
            ins=[zwd[:]],
            outs=[zwo[:]],
        )

        _dmy_n = [0]

        def warm_burst(n):
            # keep the PE HAM clock-gate at full rate while real matmuls are
            # DMA-gated; results are discarded
            for _ in range(n):
                i = _dmy_n[0]
                _dmy_n[0] += 1
                ps = psum.tile(
                    [128, 512], F32, tag=PTAGS[i % 2], bufs=2, name=f"dmy{i}"
                )
                nc.tensor.matmul(ps[:], dmy[:, 0:128], dmy[:], start=True, stop=True)

        # --- qk projection for one pair: d-outer over all 8 (q/k, chunk)
        # PSUM tiles so matmuls fire as each (w, xT) d-block arrives.
        # PSUM->SBUF copies go to the scalar engine (idle during proj). ---
        def emit_qk_phase(pair):
            tiles = {}
            order = [(nm, jc) for nm in ("q", "k") for jc in range(4)]
            for i, (nm, jc) in enumerate(order):
                tiles[(nm, jc)] = psum.tile(
                    [128, QCH], F32, tag=PTAGS[i % 4], bufs=2, name=f"pp{pair}{nm}{jc}"
                )
            for d in range(NDB):
                for (nm, jc), ps in tiles.items():
                    nc.tensor.matmul(
                        ps[:],
                        w_sb[nm][:, d * EH + 128 * pair : d * EH + 128 * pair + 128],
                        xt_sb[d][:, jc * QCH : (jc + 1) * QCH],
                        start=(d == 0),
                        stop=(d == NDB - 1),
                    )
            for (nm, jc), ps in tiles.items():
                dst = (qt if nm == "q" else kt)[pair]
                nc.scalar.copy(dst[:, jc * QCH : (jc + 1) * QCH], ps[:])

        # --- V projection for one 128-key block (copies on the DVE) ---
        def emit_proj_v(sb_i):
            ps = psum.tile(
                [128, EH], F32, tag=PTAGS[sb_i % 4], bufs=2, name=f"pv{sb_i}"
            )
            for d in range(NDB):
                nc.tensor.matmul(
                    ps[:],
                    xt_sb[d][:, sb_i * KB : (sb_i + 1) * KB],
                    w_sb["v"][:, d * EH : (d + 1) * EH],
                    start=(d == 0),
                    stop=(d == NDB - 1),
                )
            for h in range(H_LOC):
                nc.vector.tensor_copy(
                    vg[h][:, sb_i * VW : sb_i * VW + DH],
                    ps[:, h * DH : (h + 1) * DH],
                )

        # --- wo blocks + bias (gpsimd queue, spread across attention) ---
        def emit_wo_block(f):
            nc.gpsimd.dma_start(
                wo_sb[:, f * D : (f + 1) * D], wo_d[f * 128 : (f + 1) * 128, :]
            )

        # --- attention for pair p, query chunk j: both heads packed.
        # Score matmuls of the two heads are adjacent -> concurrent PE
        # row-groups. AV lags the scores by one key-block so the PE never
        # stalls on the exp engines. ---
        def emit_av(p, j, nkb, kb, c0, eA, eB, potA, potB):
            nc.tensor.matmul(
                potA[:, c0:QCH],
                vg[2 * p][:, kb * VW : (kb + 1) * VW],
                eA[:, c0:QCH],
                start=(kb == 0),
                stop=(kb == nkb - 1),
            )
            nc.tensor.matmul(
                potB[:, c0:QCH],
                vg[2 * p + 1][:, kb * VW : (kb + 1) * VW],
                eB[:, c0:QCH],
                start=(kb == 0),
                stop=(kb == nkb - 1),
            )

        def emit_attn_pair_chunk(p, j):
            nkb = 4 * (j + 1)
            potA = psum.tile([VW, QCH], F32, tag="c", bufs=2, name=f"potA{p}_{j}")
            potB = psum.tile([VW, QCH], F32, tag="d", bufs=2, name=f"potB{p}_{j}")
            pend = None
            for kb in range(nkb):
                c0 = max(0, KB * kb - QCH * j)
                pssA = psum.tile([128, QCH], F32, tag="a", bufs=2, name=f"psA{p}{j}{kb}")
                pssB = psum.tile([128, QCH], F32, tag="b", bufs=2, name=f"psB{p}{j}{kb}")
                nc.tensor.matmul(
                    pssA[:, c0:QCH],
                    kt[p][0:DH, kb * KB : (kb + 1) * KB],
                    qt[p][0:DH, j * QCH + c0 : (j + 1) * QCH],
                    start=True,
                    stop=True,
                )
                nc.tensor.matmul(
                    pssB[:, c0:QCH],
                    kt[p][DH : 2 * DH, kb * KB : (kb + 1) * KB],
                    qt[p][DH : 2 * DH, j * QCH + c0 : (j + 1) * QCH],
                    start=True,
                    stop=True,
                )
                if pend is not None:
                    emit_av(p, j, nkb, *pend, potA, potB)
                eA = work.tile([128, QCH], MM_DT, tag="eA", bufs=3, name=f"eA{p}{j}{kb}")
                eB = work.tile([128, QCH], MM_DT, tag="eB", bufs=3, name=f"eB{p}{j}{kb}")
                # head A: Schraudolph exp on DVE; head B: table exp on ACT
                nc.vector.tensor_scalar(
                    eA[:, c0:QCH].bitcast(I16),
                    pssA[:, c0:QCH],
                    A_EXP,
                    B_EXP,
                    op0=MULT,
                    op1=ADD,
                )
                nc.scalar.activation(
                    eB[:, c0:QCH], pssB[:, c0:QCH], EXP, scale=SCALE
                )
                m = kb - 4 * j
                if 0 <= m <= 3:  # diagonal sub-block: zero the upper triangle
                    ct = 128 * m
                    for e_t in (eA, eB):
                        nc.vector.tensor_tensor(
                            e_t[:, ct : ct + 128], e_t[:, ct : ct + 128], tri[:], op=MULT
                        )
                pend = (kb, c0, eA, eB)
            emit_av(p, j, nkb, *pend, potA, potB)
            # normalize both heads: oft_own = pot[0:64] / pot[64].
            # (den staged through SBUF: custom-DVE reciprocal can't read PSUM)
            den = work.tile([1, 2 * QCH], F32, tag="den", bufs=2, name=f"den{p}_{j}")
            nc.vector.tensor_copy(den[:, 0:QCH], potA[DH : DH + 1, :])
            nc.vector.tensor_copy(den[:, QCH : 2 * QCH], potB[DH : DH + 1, :])
            rec = work.tile([1, 2 * QCH], F32, tag="rec", bufs=2, name=f"rec{p}_{j}")
            nc.vector.reciprocal_approx_fast(rec[:], den[:])
            pb = work.tile([DH, 2 * QCH], F32, tag="pb", bufs=2, name=f"pb{p}_{j}")
            nc.gpsimd.partition_broadcast(pb[:], rec[0:1, :])
            nc.vector.tensor_tensor(
                oft_own[p][0:DH, j * QCH : (j + 1) * QCH],
                potA[0:DH, :],
                pb[:, 0:QCH],
                op=MULT,
            )
            nc.vector.tensor_tensor(
                oft_own[p][DH : 2 * DH, j * QCH : (j + 1) * QCH],
                potB[0:DH, :],
                pb[:, QCH : 2 * QCH],
                op=MULT,
            )

        # --- A2A plumbing ---
        a2a_bufs = {}

        def emit_a2a_cin(p, j):
            if p not in a2a_bufs:
                cin = dram.tile([N_CORES * 128, QL], MM_DT, name=f"cin{p}")
                cout = dram.tile([N_CORES * 128, QL], MM_DT, name=f"cout{p}")
                a2a_bufs[p] = (cin, cout)
            cin = a2a_bufs[p][0]
            for s in (2 * j, 2 * j + 1):
                nc.scalar.dma_start(
                    cin[s * 128 : (s + 1) * 128, :],
                    oft_own[p][:, s * QL : (s + 1) * QL],
                )

        def emit_a2a_trigger(p):
            cin, cout = a2a_bufs[p]
            nc.gpsimd.collective_compute(
                "AllToAll",
                mybir.AluOpType.bypass,
                replica_groups=[list(range(N_CORES))],
                ins=[cin[:]],
                outs=[cout[:]],
            )

        def emit_a2a_post(p):
            cin, cout = a2a_bufs[p]
            for rr in range(GROUPS):
                for bi in range(2):
                    src_rank = bi * GROUPS + rr
                    nc.sync.dma_start(
                        oft_all[2 * rr + p][:, bi * QL : (bi + 1) * QL],
                        cout[src_rank * 128 : (src_rank + 1) * 128, :],
                    )

        # ===== emission schedule =====
        warm_burst(10)
        emit_qk_phase(0)
        emit_qk_phase(1)
        for sb_i in range(NKB):
            emit_proj_v(sb_i)

        emit_attn_pair_chunk(0, 0)
        emit_a2a_cin(0, 0)
        emit_wo_block(0)
        emit_attn_pair_chunk(0, 1)
        emit_a2a_cin(0, 1)
        emit_wo_block(1)
        emit_attn_pair_chunk(0, 2)
        emit_a2a_cin(0, 2)
        emit_wo_block(2)
        emit_attn_pair_chunk(0, 3)
        emit_a2a_cin(0, 3)
        emit_a2a_trigger(0)
        emit_wo_block(3)

        emit_attn_pair_chunk(1, 3)
        emit_a2a_cin(1, 3)
        emit_wo_block(4)
        emit_wo_block(5)
        emit_attn_pair_chunk(1, 2)
        emit_a2a_cin(1, 2)
        emit_wo_block(6)
        emit_wo_block(7)
        emit_attn_pair_chunk(1, 1)
        emit_a2a_cin(1, 1)
        nc.gpsimd.dma_start(bb_sb[:], bb_d[:])
        emit_attn_pair_chunk(1, 0)
        emit_a2a_cin(1, 0)
        emit_a2a_post(0)
        emit_a2a_trigger(1)
        emit_a2a_post(1)

        # --- output projection on local 256-query slice of each batch ---
        # 8 PSUM slots (4 tiles x 2 column halves) held through both phases.
        out_tiles = [(0, 0), (0, 1), (1, 0), (1, 1)]  # (bi, qb)
        pys = [
            psum.tile([128, QCH], F32, tag=PTAGS[t], bufs=2, name=f"py{t}_{ech}")
            for t in range(4)
            for ech in range(2)
        ]

        # Gate matmuls: zero contribution (moving operand is the zeros tile),
        # but the stationary operand reads oft_own[1] cols 0:128 — written by
        # the LAST attention normalize (pair 1, chunk 0). Deliberate fence:
        # the PE queue is in-order, so without it the scheduler hoists these
        # out-proj accumulations (which wait on the collective's DMAs) into
        # the attention stream and stalls attention behind the A2A.
        for i in range(8):
            nc.tensor.matmul(
                pys[i][:], oft_own[1][:, 0:128], dmy[:], start=True, stop=False
            )

        def emit_out_phase(fs, last):
            for t, (bi, qb) in enumerate(out_tiles):
                for fi, f in enumerate(fs):
                    for ech in range(2):
                        nc.tensor.matmul(
                            pys[2 * t + ech][:],
                            oft_all[f][:, bi * QL + qb * 128 : bi * QL + (qb + 1) * 128],
                            wo_sb[:, f * D + ech * QCH : f * D + (ech + 1) * QCH],
                            start=False,
                            stop=(last and fi == len(fs) - 1),
                        )
                if last:  # per-tile drain: bias add + y DMA on rotating queues
                    ysb = work.tile([128, D], F32, tag="ysb", bufs=2, name=f"y{t}")
                    for ech in range(2):
                        nc.vector.tensor_tensor(
                            ysb[:, ech * QCH : (ech + 1) * QCH],
                            pys[2 * t + ech][:],
                            bb_sb[:, ech * QCH : (ech + 1) * QCH],
                            op=ADD,
                        )
                    row0 = bi * QL + qb * 128
                    for half in range(2):
                        qs[half % 2].dma_start(
                            y_d[row0 : row0 + 128, half * QCH : (half + 1) * QCH],
                            ysb[:, half * QCH : (half + 1) * QCH],
                        )

        emit_out_phase([0, 2, 4, 6], last=False)
        emit_out_phase([1, 3, 5, 7], last=True)


def build_program():
    nc = bacc.Bacc(
        "TRN2", target_bir_lowering=False, debug=False, num_devices=N_CORES
    )
    xT = nc.dram_tensor("xT", [D, S], BF16, kind="ExternalInput")
    wq = nc.dram_tensor("wq", [D, EH], BF16, kind="ExternalInput")
    wk = nc.dram_tensor("wk", [D, EH], BF16, kind="ExternalInput")
    wv = nc.dram_tensor("wv", [D, EH], BF16, kind="ExternalInput")
    wo = nc.dram_tensor("wo", [D, D], BF16, kind="ExternalInput")
    bb = nc.dram_tensor("bb", [128, D], F32, kind="ExternalInput")
    y = nc.dram_tensor("y", [2 * QL, D], F32, kind="ExternalOutput")
    with tile.TileContext(nc) as tc:
        _emit(nc, tc, xT.ap(), wq.ap(), wk.ap(), wv.ap(), wo.ap(), bb.ap(), y.ap())
    nc.compile()
    return nc


_cached_nc = None


def _get_nc():
    global _cached_nc
    if _cached_nc is None:
        _cached_nc = build_program()
    return _cached_nc


def make_in_maps(x, w_qkv, w_out, b_out):
    import ml_dtypes

    bf16 = ml_dtypes.bfloat16
    x = np.asarray(x, np.float32).astype(bf16)
    w_qkv = np.asarray(w_qkv, np.float32).astype(bf16)
    w_out = np.ascontiguousarray(np.asarray(w_out, np.float32).astype(bf16))
    b_out = np.asarray(b_out, np.float32)
    bb = np.ascontiguousarray(np.broadcast_to(b_out, (128, D)))
    in_maps = []
    for c in range(N_CORES):
        b, g = c // GROUPS, c % GROUPS
        in_maps.append(
            {
                "xT": np.ascontiguousarray(x[b].T),
                "wq": np.ascontiguousarray(w_qkv[:, g * EH : (g + 1) * EH]),
                "wk": np.ascontiguousarray(w_qkv[:, D + g * EH : D + (g + 1) * EH]),
                "wv": np.ascontiguousarray(
                    w_qkv[:, 2 * D + g * EH : 2 * D + (g + 1) * EH]
                ),
                "wo": w_out,
                "bb": bb,
            }
        )
    return in_maps


def assemble(results):
    # core c's y is [512, D]: rows [0,256) = batch 0 q-slice [256c, 256c+256),
    # rows [256,512) = batch 1 same slice.
    y = np.empty((B, S, D), np.float32)
    for c in range(N_CORES):
        yc = results[c]["y"]
        y[0, 256 * c : 256 * (c + 1), :] = yc[:256]
        y[1, 256 * c : 256 * (c + 1), :] = yc[256:]
    return y


def kernel(x, w_qkv, w_out, b_out, _trace=False, **run_kwargs):
    nc = _get_nc()
    in_maps = make_in_maps(x, w_qkv, w_out, b_out)
    res = run_bass_kernel_spmd(
        nc, in_maps, core_ids=list(range(N_CORES)), trace=_trace, **run_kwargs
    )
    out = assemble(res.results)
    if _trace:
        return out, res
    return out


# revision 12
# speedup vs baseline: 1.2158x; 1.1833x over previous
"""Multi-head causal attention (B=2, S=2048, D=1024, H=16) on 8 NeuronCores.

Sharding: core c = (batch b=c//4, head-group g=c%4 of 4 heads).
Schedule per core:
  1. qk projection for both head pairs (d-outer over 8 PSUM banks so matmuls
     fire as the per-block weight/xT DMAs land), then V projection.
  2. Attention pair-by-pair with BOTH heads of a pair packed per key-block:
     the two K=64 score matmuls sit in PE row-groups 0-1 / 2-3 (tile_position
     auto-derived from base partitions 0/64) and run concurrently; softmax
     exp is split across engines (head A: Schraudolph int16-bitcast exp on
     the vector engine, head B: ACT-table exp on the scalar engine); the A@V
     matmuls accumulate the fused ones-column denominator (VW=65).
     AV for key-block kb is emitted after the scores of kb+1 (lag-1 software
     pipeline) so the PE never waits on the exp engines.
  3. Two 8-core AllToAlls swap head-shards for query-shards (one per pair;
     pair-0's fires at the attention midpoint). A tiny warmup AllToAll at
     kernel start absorbs the first-collective setup cost.
  4. Output projection on a fixed local 256-query slice of each batch:
     pair-0 f-blocks accumulate during the second AllToAll's flight, pair-1
     blocks after it lands; outputs drain per-tile across four DMA queues.

PSUM budget (8 banks): tags a/b/c/d, each 2 bufs of [128,512]f32 slots.
"""

import numpy as np

import concourse.bass as bass
import concourse.mybir as mybir
import concourse.tile as tile
from concourse import bacc
from concourse.bass_utils import run_bass_kernel_spmd

B, S, D = 2, 2048, 1024
H = 16
DH = 64  # head dim
N_CORES = 8
GROUPS = 4  # cores per batch = head groups
H_LOC = H // GROUPS  # 4 heads per core
EH = H_LOC * DH  # 256 local qkv width
QCH = 512  # query chunk
KB = 128  # key block
NKB = S // KB  # 16
NDB = D // 128  # 8 contraction blocks
QL = 256  # local output query rows per batch
VW = DH + 1  # 65: V columns + fused ones column
SCALE = 1.0 / 8.0  # 1/sqrt(DH)

# Schraudolph exp on the DVE: i16 = round(score*SCALE*128/ln2 + (127*128-c));
# the int16 bit pattern read as bf16 approximates exp(score*SCALE) (+-3%).
A_EXP = SCALE * 184.6650230929499  # 128/ln(2) * SCALE
B_EXP = 16248.6  # 127*128 - 7.4 (mid-point bias correction)

F32 = mybir.dt.float32
BF16 = mybir.dt.bfloat16
I16 = mybir.dt.int16
MM_DT = BF16
EXP = mybir.ActivationFunctionType.Exp
MULT = mybir.AluOpType.mult
ADD = mybir.AluOpType.add


def _emit(nc, tc, xT, wq_d, wk_d, wv_d, wo_d, bb_d, y_d):
    from contextlib import ExitStack

    ctx = ExitStack()
    with ctx:
        persist = ctx.enter_context(tc.tile_pool(name="persist", bufs=1))
        psum = ctx.enter_context(tc.tile_pool(name="psum", bufs=1, space="PSUM"))
        dram = ctx.enter_context(tc.tile_pool(name="dram", bufs=1, space="DRAM"))
        work = ctx.enter_context(tc.tile_pool(name="work", bufs=1))

        PTAGS = ["a", "b", "c", "d"]

        # --- constants ---
        ones_f = persist.tile([128, 1], F32)
        nc.vector.memset(ones_f[:], 1.0)
        dmy = persist.tile([128, 512], MM_DT)
        nc.vector.memset(dmy[:], 0.0)
        # tri[k, t] = 1 if t >= k else 0 (bf16): causal mask for a diagonal
        # 128-key x 128-query sub-block. (gpsimd: affine_select lives there.)
        tri = persist.tile([128, 128], MM_DT)
        nc.gpsimd.memset(tri[:], 1.0)
        nc.gpsimd.affine_select(
            out=tri[:],
            in_=tri[:],
            compare_op=mybir.AluOpType.is_ge,
            fill=0.0,
            base=0,
            channel_multiplier=-1,
            pattern=[[1, 128]],
        )
        bb_sb = persist.tile([128, D], F32)

        # --- persistent operand tiles ---
        xt_sb = [persist.tile([128, S], MM_DT, name=f"xt{d}") for d in range(NDB)]
        w_sb = {
            nm: persist.tile([128, NDB * EH], MM_DT, name=f"w{nm}sb")
            for nm in ("q", "k", "v")
        }
        wo_sb = persist.tile([128, NDB * D], MM_DT)
        qt = [persist.tile([128, S], MM_DT, name=f"qt{p}") for p in range(2)]
        kt = [persist.tile([128, S], MM_DT, name=f"kt{p}") for p in range(2)]
        vg = [persist.tile([128, NKB * VW], MM_DT, name=f"vg{h}") for h in range(H_LOC)]
        for h in range(H_LOC):
            nc.vector.tensor_copy(
                vg[h].rearrange("p (n w) -> p n w", w=VW)[:, :, DH : DH + 1],
                ones_f[:].unsqueeze(2).broadcast_to([128, NKB, 1]),
            )
        oft_own = [persist.tile([128, S], MM_DT, name=f"oftown{p}") for p in range(2)]
        oft_all = [persist.tile([128, 2 * QL], MM_DT, name=f"oft{f}") for f in range(NDB)]

        # --- input DMAs: per-d-block, round-robin over the DMA-capable
        # queues (sync/scalar) so the d-outer projection only waits for the
        # blocks it needs next; wv on gpsimd (needed later, at V-proj) ---
        qs = [nc.sync, nc.scalar]
        for d in range(NDB):
            q = qs[d % 2]
            q.dma_start(w_sb["q"][:, d * EH : (d + 1) * EH], wq_d[d * 128 : (d + 1) * 128, :])
            q.dma_start(w_sb["k"][:, d * EH : (d + 1) * EH], wk_d[d * 128 : (d + 1) * 128, :])
            q.dma_start(xt_sb[d][:], xT[d * 128 : (d + 1) * 128, :])
        for d in range(NDB):
            nc.gpsimd.dma_start(
                w_sb["v"][:, d * EH : (d + 1) * EH], wv_d[d * 128 : (d + 1) * 128, :]
            )

        # --- warmup AllToAll: absorbs the first-collective setup cost on the
        # CC core so the real pair-0 AllToAll starts promptly ---
        zw = persist.tile([8, 16], MM_DT)
        nc.vector.memset(zw[:], 0.0)
        zwd = dram.tile([8, 16], MM_DT, name="zwd")
        zwo = dram.tile([8, 16], MM_DT, name="zwo")
        nc.sync.dma_start(zwd[:], zw[:])
        nc.gpsimd.collective_compute(
            "AllToAll",
            mybir.AluOpType.bypass,
            replica_groups=[list(range(N_CORES))],
            ins=[zwd[:]],
            outs=[zwo[:]],
        )

        _dmy_n = [0]

        def warm_burst(n):
            # keep the PE HAM clock-gate at full rate while real matmuls are
            # DMA-gated; results are discarded
            for _ in range(n):
                i = _dmy_n[0]
                _dmy_n[0] += 1
                ps = psum.tile(
                    [128, 512], F32, tag=PTAGS[i % 2], bufs=2, name=f"dmy{i}"
                )
                nc.tensor.matmul(ps[:], dmy[:, 0:128], dmy[:], start=True, stop=True)

        # --- qk projection for one pair: d-outer over all 8 (q/k, chunk)
        # PSUM tiles so matmuls fire as each (w, xT) d-block arrives.
        # PSUM->SBUF copies go to the scalar engine (idle during proj). ---
        def emit_qk_phase(pair):
            tiles = {}
            order = [(nm, jc) for nm in ("q", "k") for jc in range(4)]
            for i, (nm, jc) in enumerate(order):
                tiles[(nm, jc)] = psum.tile(
                    [128, QCH], F32, tag=PTAGS[i % 4], bufs=2, name=f"pp{pair}{nm}{jc}"
                )
            for d in range(NDB):
                for (nm, jc), ps in tiles.items():
                    nc.tensor.matmul(
                        ps[:],
                        w_sb[nm][:, d * EH + 128 * pair : d * EH + 128 * pair + 128],
                        xt_sb[d][:, jc * QCH : (jc + 1) * QCH],
                        start=(d == 0),
                        stop=(d == NDB - 1),
                    )
            for (nm, jc), ps in tiles.items():
                dst = (qt if nm == "q" else kt)[pair]
                nc.scalar.copy(dst[:, jc * QCH : (jc + 1) * QCH], ps[:])

        # --- V projection for one 128-key block (copies on the DVE) ---
        def emit_proj_v(sb_i):
            ps = psum.tile(
                [128, EH], F32, tag=PTAGS[sb_i % 4], bufs=2, name=f"pv{sb_i}"
            )
            for d in range(NDB):
                nc.tensor.matmul(
                    ps[:],
                    xt_sb[d][:, sb_i * KB : (sb_i + 1) * KB],
                    w_sb["v"][:, d * EH : (d + 1) * EH],
                    start=(d == 0),
                    stop=(d == NDB - 1),
                )
            for h in range(H_LOC):
                nc.vector.tensor_copy(
                    vg[h][:, sb_i * VW : sb_i * VW + DH],
                    ps[:, h * DH : (h + 1) * DH],
                )

        # --- wo blocks + bias (gpsimd queue, spread across attention) ---
        def emit_wo_block(f):
            nc.gpsimd.dma_start(
                wo_sb[:, f * D : (f + 1) * D], wo_d[f * 128 : (f + 1) * 128, :]
            )

        # --- attention for pair p: both heads packed, and TWO query chunks
        # interleaved kb-outer so (a) each key-block's stationary kt/vg
        # LDWEIGHTS is shared by both chunks, (b) AV matmuls lag the scores
        # by one key-block, giving the exp engines a full step of slack, and
        # (c) the PE stream stays dense (HAM stays at full clock).
        # Score matmuls of the two heads sit in different PE row-groups
        # (base partitions 0/64) and run concurrently. ---
        def emit_attn_sweep(p, jhi, jlo):
            pots = {}
            for j in (jhi, jlo):
                pots[j] = (
                    psum.tile([VW, QCH], F32, tag="c", bufs=2, name=f"potA{p}_{j}"),
                    psum.tile([VW, QCH], F32, tag="d", bufs=2, name=f"potB{p}_{j}"),
                )

            def flush_avs(pend):
                # head-major so the vg LDWEIGHTS is shared across chunks
                for hh in range(2):
                    for j, kb, c0, es in pend:
                        nc.tensor.matmul(
                            pots[j][hh][:, c0:QCH],
                            vg[2 * p + hh][:, kb * VW : (kb + 1) * VW],
                            es[hh][:, c0:QCH],
                            start=(kb == 0),
                            stop=(kb == 4 * (j + 1) - 1),
                        )

            pend = []
            for kb in range(4 * (jhi + 1)):
                js = [j for j in (jhi, jlo) if kb < 4 * (j + 1)]
                step = []
                for hh, tg in ((0, "a"), (1, "b")):
                    for j in js:
                        c0 = max(0, KB * kb - QCH * j)
                        ps = psum.tile(
                            [128, QCH], F32, tag=tg, bufs=2, name=f"ps{hh}{p}{j}{kb}"
                        )
                        nc.tensor.matmul(
                            ps[:, c0:QCH],
                            kt[p][hh * DH : (hh + 1) * DH, kb * KB : (kb + 1) * KB],
                            qt[p][hh * DH : (hh + 1) * DH, j * QCH + c0 : (j + 1) * QCH],
                            start=True,
                            stop=True,
                        )
                        step.append((hh, j, c0, ps))
                flush_avs(pend)
                pend = []
                byj = {}
                for hh, j, c0, ps in step:
                    es = byj.setdefault(j, [None, None, c0])
                    e_t = work.tile(
                        [128, QCH], MM_DT, tag=f"e{hh}", bufs=5, name=f"e{hh}{p}{j}{kb}"
                    )
                    es[hh] = e_t
                    if hh == 0:  # head A: Schraudolph exp on DVE
                        nc.vector.tensor_scalar(
                            e_t[:, c0:QCH].bitcast(I16),
                            ps[:, c0:QCH],
                            A_EXP,
                            B_EXP,
                            op0=MULT,
                            op1=ADD,
                        )
                    else:  # head B: table exp on ACT
                        nc.scalar.activation(
                            e_t[:, c0:QCH], ps[:, c0:QCH], EXP, scale=SCALE
                        )
                    m = kb - 4 * j
                    if 0 <= m <= 3:  # diagonal sub-block: zero the upper triangle
                        ct = 128 * m
                        nc.vector.tensor_tensor(
                            e_t[:, ct : ct + 128], e_t[:, ct : ct + 128], tri[:], op=MULT
                        )
                for j, (eA, eB, c0) in byj.items():
                    pend.append((j, kb, c0, (eA, eB)))
            flush_avs(pend)
            for j in (jhi, jlo):
                emit_normalize(p, j, *pots[j])
                emit_a2a_cin(p, j)

        def emit_normalize(p, j, potA, potB):
            # oft_own = pot[0:64] / pot[64].
            # (den staged through SBUF: custom-DVE reciprocal can't read PSUM)
            den = work.tile([1, 2 * QCH], F32, tag="den", bufs=2, name=f"den{p}_{j}")
            nc.vector.tensor_copy(den[:, 0:QCH], potA[DH : DH + 1, :])
            nc.vector.tensor_copy(den[:, QCH : 2 * QCH], potB[DH : DH + 1, :])
            rec = work.tile([1, 2 * QCH], F32, tag="rec", bufs=2, name=f"rec{p}_{j}")
            nc.vector.reciprocal_approx_fast(rec[:], den[:])
            pb = work.tile([DH, 2 * QCH], F32, tag="pb", bufs=2, name=f"pb{p}_{j}")
            nc.gpsimd.partition_broadcast(pb[:], rec[0:1, :])
            nc.vector.tensor_tensor(
                oft_own[p][0:DH, j * QCH : (j + 1) * QCH],
                potA[0:DH, :],
                pb[:, 0:QCH],
                op=MULT,
            )
            nc.vector.tensor_tensor(
                oft_own[p][DH : 2 * DH, j * QCH : (j + 1) * QCH],
                potB[0:DH, :],
                pb[:, QCH : 2 * QCH],
                op=MULT,
            )

        # --- A2A plumbing ---
        a2a_bufs = {}

        def emit_a2a_cin(p, j):
            if p not in a2a_bufs:
                cin = dram.tile([N_CORES * 128, QL], MM_DT, name=f"cin{p}")
                cout = dram.tile([N_CORES * 128, QL], MM_DT, name=f"cout{p}")
                a2a_bufs[p] = (cin, cout)
            cin = a2a_bufs[p][0]
            for s in (2 * j, 2 * j + 1):
                nc.sync.dma_start(
                    cin[s * 128 : (s + 1) * 128, :],
                    oft_own[p][:, s * QL : (s + 1) * QL],
                )

        def emit_a2a_trigger(p):
            cin, cout = a2a_bufs[p]
            nc.gpsimd.collective_compute(
                "AllToAll",
                mybir.AluOpType.bypass,
                replica_groups=[list(range(N_CORES))],
                ins=[cin[:]],
                outs=[cout[:]],
            )

        def emit_a2a_post(p):
            cin, cout = a2a_bufs[p]
            for rr in range(GROUPS):
                for bi in range(2):
                    src_rank = bi * GROUPS + rr
                    nc.sync.dma_start(
                        oft_all[2 * rr + p][:, bi * QL : (bi + 1) * QL],
                        cout[src_rank * 128 : (src_rank + 1) * 128, :],
                    )

        # ===== emission schedule =====
        warm_burst(10)
        emit_qk_phase(0)
        emit_qk_phase(1)
        for sb_i in range(NKB):
            emit_proj_v(sb_i)

        emit_attn_sweep(0, 3, 2)
        emit_wo_block(0)
        emit_wo_block(1)
        emit_attn_sweep(0, 1, 0)
        emit_a2a_trigger(0)
        emit_wo_block(2)
        emit_wo_block(3)

        emit_attn_sweep(1, 3, 2)
        emit_wo_block(4)
        emit_wo_block(5)
        emit_wo_block(6)
        emit_wo_block(7)
        nc.gpsimd.dma_start(bb_sb[:], bb_d[:])
        emit_attn_sweep(1, 1, 0)
        emit_a2a_post(0)
        emit_a2a_trigger(1)
        emit_a2a_post(1)

        # --- output projection on local 256-query slice of each batch ---
        # 8 PSUM slots (4 tiles x 2 column halves) held through both phases.
        out_tiles = [(0, 0), (0, 1), (1, 0), (1, 1)]  # (bi, qb)
        pys = [
            psum.tile([128, QCH], F32, tag=PTAGS[t], bufs=2, name=f"py{t}_{ech}")
            for t in range(4)
            for ech in range(2)
        ]

        # Gate matmuls: zero contribution (moving operand is the zeros tile),
        # but the stationary operand reads oft_own[1] cols 0:128 — written by
        # the LAST attention normalize (pair 1, chunk 0). Deliberate fence:
        # the PE queue is in-order, so without it the scheduler hoists these
        # out-proj accumulations (which wait on the collective's DMAs) into
        # the attention stream and stalls attention behind the A2A.
        for i in range(8):
            nc.tensor.matmul(
                pys[i][:], oft_own[1][:, 0:128], dmy[:], start=True, stop=False
            )

        def emit_out_phase(fs, last):
            for t, (bi, qb) in enumerate(out_tiles):
                for fi, f in enumerate(fs):
                    for ech in range(2):
                        nc.tensor.matmul(
                            pys[2 * t + ech][:],
                            oft_all[f][:, bi * QL + qb * 128 : bi * QL + (qb + 1) * 128],
                            wo_sb[:, f * D + ech * QCH : f * D + (ech + 1) * QCH],
                            start=False,
                            stop=(last and fi == len(fs) - 1),
                        )
                if last:  # per-tile drain: bias add + y DMA on rotating queues
                    ysb = work.tile([128, D], F32, tag="ysb", bufs=2, name=f"y{t}")
                    for ech in range(2):
                        nc.vector.tensor_tensor(
                            ysb[:, ech * QCH : (ech + 1) * QCH],
                            pys[2 * t + ech][:],
                            bb_sb[:, ech * QCH : (ech + 1) * QCH],
                            op=ADD,
                        )
                    row0 = bi * QL + qb * 128
                    for half in range(2):
                        qs[half % 2].dma_start(
                            y_d[row0 : row0 + 128, half * QCH : (half + 1) * QCH],
                            ysb[:, half * QCH : (half + 1) * QCH],
                        )

        emit_out_phase([0, 2, 4, 6], last=False)
        emit_out_phase([1, 3, 5, 7], last=True)


def build_program():
    nc = bacc.Bacc(
        "TRN2", target_bir_lowering=False, debug=False, num_devices=N_CORES
    )
    xT = nc.dram_tensor("xT", [D, S], BF16, kind="ExternalInput")
    wq = nc.dram_tensor("wq", [D, EH], BF16, kind="ExternalInput")
    wk = nc.dram_tensor("wk", [D, EH], BF16, kind="ExternalInput")
    wv = nc.dram_tensor("wv", [D, EH], BF16, kind="ExternalInput")
    wo = nc.dram_tensor("wo", [D, D], BF16, kind="ExternalInput")
    bb = nc.dram_tensor("bb", [128, D], F32, kind="ExternalInput")
    y = nc.dram_tensor("y", [2 * QL, D], F32, kind="ExternalOutput")
    with tile.TileContext(nc) as tc:
        _emit(nc, tc, xT.ap(), wq.ap(), wk.ap(), wv.ap(), wo.ap(), bb.ap(), y.ap())
    nc.compile()
    return nc


_cached_nc = None


def _get_nc():
    global _cached_nc
    if _cached_nc is None:
        _cached_nc = build_program()
    return _cached_nc


def make_in_maps(x, w_qkv, w_out, b_out):
    import ml_dtypes

    bf16 = ml_dtypes.bfloat16
    x = np.asarray(x, np.float32).astype(bf16)
    w_qkv = np.asarray(w_qkv, np.float32).astype(bf16)
    w_out = np.ascontiguousarray(np.asarray(w_out, np.float32).astype(bf16))
    b_out = np.asarray(b_out, np.float32)
    bb = np.ascontiguousarray(np.broadcast_to(b_out, (128, D)))
    in_maps = []
    for c in range(N_CORES):
        b, g = c // GROUPS, c % GROUPS
        in_maps.append(
            {
                "xT": np.ascontiguousarray(x[b].T),
                "wq": np.ascontiguousarray(w_qkv[:, g * EH : (g + 1) * EH]),
                "wk": np.ascontiguousarray(w_qkv[:, D + g * EH : D + (g + 1) * EH]),
                "wv": np.ascontiguousarray(
                    w_qkv[:, 2 * D + g * EH : 2 * D + (g + 1) * EH]
                ),
                "wo": w_out,
                "bb": bb,
            }
        )
    return in_maps


def assemble(results):
    # core c's y is [512, D]: rows [0,256) = batch 0 q-slice [256c, 256c+256),
    # rows [256,512) = batch 1 same slice.
    y = np.empty((B, S, D), np.float32)
    for c in range(N_CORES):
        yc = results[c]["y"]
        y[0, 256 * c : 256 * (c + 1), :] = yc[:256]
        y[1, 256 * c : 256 * (c + 1), :] = yc[256:]
    return y


def kernel(x, w_qkv, w_out, b_out, _trace=False, **run_kwargs):
    nc = _get_nc()
    in_maps = make_in_maps(x, w_qkv, w_out, b_out)
    res = run_bass_kernel_spmd(
        nc, in_maps, core_ids=list(range(N_CORES)), trace=_trace, **run_kwargs
    )
    out = assemble(res.results)
    if _trace:
        return out, res
    return out


# revision 18
# speedup vs baseline: 1.2391x; 1.0191x over previous
"""Multi-head causal attention (B=2, S=2048, D=1024, H=16) on 8 NeuronCores.

Sharding: core c = (batch b=c//4, head-group g=c%4 of 4 heads).
Schedule per core:
  1. qk projection for both head pairs (d-outer over 8 PSUM banks so matmuls
     fire as the per-block weight/xT DMAs land), then V projection.
  2. Attention pair-by-pair with BOTH heads of a pair packed per key-block:
     the two K=64 score matmuls sit in PE row-groups 0-1 / 2-3 (tile_position
     auto-derived from base partitions 0/64) and run concurrently; softmax
     exp is split across engines (head A: Schraudolph int16-bitcast exp on
     the vector engine, head B: ACT-table exp on the scalar engine); the A@V
     matmuls accumulate the fused ones-column denominator (VW=65).
     AV for key-block kb is emitted after the scores of kb+1 (lag-1 software
     pipeline) so the PE never waits on the exp engines.
  3. Two 8-core AllToAlls swap head-shards for query-shards (one per pair;
     pair-0's fires at the attention midpoint). A tiny warmup AllToAll at
     kernel start absorbs the first-collective setup cost.
  4. Output projection on a fixed local 256-query slice of each batch:
     pair-0 f-blocks accumulate during the second AllToAll's flight, pair-1
     blocks after it lands; outputs drain per-tile across four DMA queues.

PSUM budget (8 banks): tags a/b/c/d, each 2 bufs of [128,512]f32 slots.
"""

import numpy as np

import concourse.bass as bass
import concourse.mybir as mybir
import concourse.tile as tile
from concourse import bacc
from concourse.bass_utils import run_bass_kernel_spmd

B, S, D = 2, 2048, 1024
H = 16
DH = 64  # head dim
N_CORES = 8
GROUPS = 4  # cores per batch = head groups
H_LOC = H // GROUPS  # 4 heads per core
EH = H_LOC * DH  # 256 local qkv width
QCH = 512  # query chunk
KB = 128  # key block
NKB = S // KB  # 16
NDB = D // 128  # 8 contraction blocks
QL = 256  # local output query rows per batch
VW = DH + 1  # 65: V columns + fused ones column
SCALE = 1.0 / 8.0  # 1/sqrt(DH)

# Schraudolph exp on the DVE: i16 = round(score*SCALE*128/ln2 + (127*128-c));
# the int16 bit pattern read as bf16 approximates exp(score*SCALE) (+-3%).
A_EXP = SCALE * 184.6650230929499  # 128/ln(2) * SCALE
B_EXP = 16248.6  # 127*128 - 7.4 (mid-point bias correction)

F32 = mybir.dt.float32
BF16 = mybir.dt.bfloat16
I16 = mybir.dt.int16
MM_DT = BF16
EXP = mybir.ActivationFunctionType.Exp
MULT = mybir.AluOpType.mult
ADD = mybir.AluOpType.add


def _emit(nc, tc, xT, wq_d, wk_d, wv_d, wo_d, bb_d, y_d):
    from contextlib import ExitStack

    ctx = ExitStack()
    with ctx:
        persist = ctx.enter_context(tc.tile_pool(name="persist", bufs=1))
        psum = ctx.enter_context(tc.tile_pool(name="psum", bufs=1, space="PSUM"))
        dram = ctx.enter_context(tc.tile_pool(name="dram", bufs=1, space="DRAM"))
        work = ctx.enter_context(tc.tile_pool(name="work", bufs=1))

        PTAGS = ["a", "b", "c", "d"]

        # --- constants ---
        ones_f = persist.tile([128, 1], F32)
        nc.vector.memset(ones_f[:], 1.0)
        dmy = persist.tile([128, 512], MM_DT)
        nc.vector.memset(dmy[:], 0.0)
        # tri[k, t] = 1 if t >= k else 0 (bf16): causal mask for a diagonal
        # 128-key x 128-query sub-block. (gpsimd: affine_select lives there.)
        tri = persist.tile([128, 128], MM_DT)
        nc.gpsimd.memset(tri[:], 1.0)
        nc.gpsimd.affine_select(
            out=tri[:],
            in_=tri[:],
            compare_op=mybir.AluOpType.is_ge,
            fill=0.0,
            base=0,
            channel_multiplier=-1,
            pattern=[[1, 128]],
        )
        bb_sb = persist.tile([128, D], F32)

        # --- persistent operand tiles ---
        xt_sb = [persist.tile([128, S], MM_DT, name=f"xt{d}") for d in range(NDB)]
        w_sb = {
            nm: persist.tile([128, NDB * EH], MM_DT, name=f"w{nm}sb")
            for nm in ("q", "k", "v")
        }
        wo_sb = persist.tile([128, NDB * D], MM_DT)
        qt = [persist.tile([128, S], MM_DT, name=f"qt{p}") for p in range(2)]
        kt = [persist.tile([128, S], MM_DT, name=f"kt{p}") for p in range(2)]
        # vg is padded so a 128-wide stationary slice starting at any block is
        # in-bounds: the AV matmuls use [128,128] stationaries (cols 65..127
        # spill into the next block = junk, accumulated into unread PSUM rows
        # 65..127) purely so FWL + background weight loading kicks in.
        VGW = NKB * VW + 128
        vg = [persist.tile([128, VGW], MM_DT, name=f"vg{h}") for h in range(H_LOC)]
        for h in range(H_LOC):
            nc.vector.memset(vg[h][:, NKB * VW : VGW], 0.0)
            nc.vector.tensor_copy(
                vg[h][:, 0 : NKB * VW].rearrange("p (n w) -> p n w", w=VW)[
                    :, :, DH : DH + 1
                ],
                ones_f[:].unsqueeze(2).broadcast_to([128, NKB, 1]),
            )
        oft_own = [persist.tile([128, S], MM_DT, name=f"oftown{p}") for p in range(2)]
        oft_all = [persist.tile([128, 2 * QL], MM_DT, name=f"oft{f}") for f in range(NDB)]

        # --- warmup AllToAll, triggered before the input-DMA storm: starts
        # the CC core's (slow, variable) init as early as possible so the
        # real pair-0 AllToAll isn't gated on it ---
        zw = persist.tile([8, 16], MM_DT)
        nc.vector.memset(zw[:], 0.0)
        zwd = dram.tile([8, 16], MM_DT, name="zwd")
        zwo = dram.tile([8, 16], MM_DT, name="zwo")
        nc.sync.dma_start(zwd[:], zw[:])
        nc.gpsimd.collective_compute(
            "AllToAll",
            mybir.AluOpType.bypass,
            replica_groups=[list(range(N_CORES))],
            ins=[zwd[:]],
            outs=[zwo[:]],
        )

        # --- input DMAs: per-d-block, round-robin over the DMA-capable
        # queues (sync/scalar) so the d-outer projection only waits for the
        # blocks it needs next; wv on gpsimd (needed later, at V-proj) ---
        qs = [nc.sync, nc.scalar]
        for d in range(NDB):
            q = qs[d % 2]
            q.dma_start(w_sb["q"][:, d * EH : (d + 1) * EH], wq_d[d * 128 : (d + 1) * 128, :])
            q.dma_start(w_sb["k"][:, d * EH : (d + 1) * EH], wk_d[d * 128 : (d + 1) * 128, :])
            q.dma_start(xt_sb[d][:], xT[d * 128 : (d + 1) * 128, :])
        for d in range(NDB):
            nc.gpsimd.dma_start(
                w_sb["v"][:, d * EH : (d + 1) * EH], wv_d[d * 128 : (d + 1) * 128, :]
            )

        _dmy_n = [0]

        def warm_burst(n):
            # keep the PE HAM clock-gate at full rate while real matmuls are
            # DMA-gated; results are discarded
            for _ in range(n):
                i = _dmy_n[0]
                _dmy_n[0] += 1
                ps = psum.tile(
                    [128, 512], F32, tag=PTAGS[i % 2], bufs=2, name=f"dmy{i}"
                )
                nc.tensor.matmul(ps[:], dmy[:, 0:128], dmy[:], start=True, stop=True)

        # --- qk projection for one pair: d-outer over all 8 (q/k, chunk)
        # PSUM tiles so matmuls fire as each (w, xT) d-block arrives.
        # PSUM->SBUF copies go to the scalar engine (idle during proj). ---
        def emit_qk_phase(pair):
            tiles = {}
            order = [(nm, jc) for nm in ("q", "k") for jc in range(4)]
            for i, (nm, jc) in enumerate(order):
                tiles[(nm, jc)] = psum.tile(
                    [128, QCH], F32, tag=PTAGS[i % 4], bufs=2, name=f"pp{pair}{nm}{jc}"
                )
            for d in range(NDB):
                for (nm, jc), ps in tiles.items():
                    nc.tensor.matmul(
                        ps[:],
                        w_sb[nm][:, d * EH + 128 * pair : d * EH + 128 * pair + 128],
                        xt_sb[d][:, jc * QCH : (jc + 1) * QCH],
                        start=(d == 0),
                        stop=(d == NDB - 1),
                    )
            for (nm, jc), ps in tiles.items():
                dst = (qt if nm == "q" else kt)[pair]
                nc.scalar.copy(dst[:, jc * QCH : (jc + 1) * QCH], ps[:])

        # --- V projection for one 128-key block (copies on the DVE) ---
        def emit_proj_v(sb_i):
            ps = psum.tile(
                [128, EH], F32, tag=PTAGS[sb_i % 4], bufs=2, name=f"pv{sb_i}"
            )
            for d in range(NDB):
                nc.tensor.matmul(
                    ps[:],
                    xt_sb[d][:, sb_i * KB : (sb_i + 1) * KB],
                    w_sb["v"][:, d * EH : (d + 1) * EH],
                    start=(d == 0),
                    stop=(d == NDB - 1),
                )
            for h in range(H_LOC):
                nc.vector.tensor_copy(
                    vg[h][:, sb_i * VW : sb_i * VW + DH],
                    ps[:, h * DH : (h + 1) * DH],
                )

        # --- wo blocks + bias (gpsimd queue, spread across attention) ---
        def emit_wo_block(f):
            nc.gpsimd.dma_start(
                wo_sb[:, f * D : (f + 1) * D], wo_d[f * 128 : (f + 1) * 128, :]
            )

        # --- attention for pair p: both heads packed, and TWO query chunks
        # interleaved kb-outer so (a) each key-block's stationary kt/vg
        # LDWEIGHTS is shared by both chunks, (b) AV matmuls lag the scores
        # by one key-block, giving the exp engines a full step of slack, and
        # (c) the PE stream stays dense (HAM stays at full clock).
        # Score matmuls of the two heads sit in different PE row-groups
        # (base partitions 0/64) and run concurrently. ---
        def emit_attn_sweep(p, jhi, jlo):
            pots = {}
            for j in (jhi, jlo):
                pots[j] = (
                    psum.tile([128, QCH], F32, tag="c", bufs=2, name=f"potA{p}_{j}"),
                    psum.tile([128, QCH], F32, tag="d", bufs=2, name=f"potB{p}_{j}"),
                )

            def flush_avs(pend):
                # head-major so the vg LDWEIGHTS is shared across chunks
                for hh in range(2):
                    for j, kb, c0, es in pend:
                        nc.tensor.matmul(
                            pots[j][hh][:, c0:QCH],
                            vg[2 * p + hh][:, kb * VW : kb * VW + 128],
                            es[hh][:, c0:QCH],
                            start=(kb == 0),
                            stop=(kb == 4 * (j + 1) - 1),
                        )

            pend = []
            for kb in range(4 * (jhi + 1)):
                js = [j for j in (jhi, jlo) if kb < 4 * (j + 1)]
                step = []
                for hh, tg in ((0, "a"), (1, "b")):
                    for j in js:
                        c0 = max(0, KB * kb - QCH * j)
                        ps = psum.tile(
                            [128, QCH], F32, tag=tg, bufs=2, name=f"ps{hh}{p}{j}{kb}"
                        )
                        nc.tensor.matmul(
                            ps[:, c0:QCH],
                            kt[p][hh * DH : (hh + 1) * DH, kb * KB : (kb + 1) * KB],
                            qt[p][hh * DH : (hh + 1) * DH, j * QCH + c0 : (j + 1) * QCH],
                            start=True,
                            stop=True,
                        )
                        step.append((hh, j, c0, ps))
                flush_avs(pend)
                pend = []
                byj = {}
                for hh, j, c0, ps in step:
                    es = byj.setdefault(j, [None, None, c0])
                    e_t = work.tile(
                        [128, QCH], MM_DT, tag=f"e{hh}", bufs=5, name=f"e{hh}{p}{j}{kb}"
                    )
                    es[hh] = e_t
                    if hh == 0:  # head A: Schraudolph exp on DVE
                        nc.vector.tensor_scalar(
                            e_t[:, c0:QCH].bitcast(I16),
                            ps[:, c0:QCH],
                            A_EXP,
                            B_EXP,
                            op0=MULT,
                            op1=ADD,
                        )
                    else:  # head B: table exp on ACT
                        nc.scalar.activation(
                            e_t[:, c0:QCH], ps[:, c0:QCH], EXP, scale=SCALE
                        )
                    m = kb - 4 * j
                    if 0 <= m <= 3:  # diagonal sub-block: zero the upper triangle
                        ct = 128 * m
                        nc.vector.tensor_tensor(
                            e_t[:, ct : ct + 128], e_t[:, ct : ct + 128], tri[:], op=MULT
                        )
                for j, (eA, eB, c0) in byj.items():
                    pend.append((j, kb, c0, (eA, eB)))
            flush_avs(pend)
            for j in (jhi, jlo):
                emit_normalize(p, j, *pots[j])
                emit_a2a_cin(p, j)

        def emit_normalize(p, j, potA, potB):
            # oft_own = pot[0:64] / pot[64].
            # (den staged through SBUF: custom-DVE reciprocal can't read PSUM)
            den = work.tile([1, 2 * QCH], F32, tag="den", bufs=2, name=f"den{p}_{j}")
            nc.vector.tensor_copy(den[:, 0:QCH], potA[DH : DH + 1, :])
            nc.vector.tensor_copy(den[:, QCH : 2 * QCH], potB[DH : DH + 1, :])
            rec = work.tile([1, 2 * QCH], F32, tag="rec", bufs=2, name=f"rec{p}_{j}")
            nc.vector.reciprocal_approx_fast(rec[:], den[:])
            pb = work.tile([DH, 2 * QCH], F32, tag="pb", bufs=2, name=f"pb{p}_{j}")
            nc.gpsimd.partition_broadcast(pb[:], rec[0:1, :])
            nc.vector.tensor_tensor(
                oft_own[p][0:DH, j * QCH : (j + 1) * QCH],
                potA[0:DH, :],
                pb[:, 0:QCH],
                op=MULT,
            )
            nc.vector.tensor_tensor(
                oft_own[p][DH : 2 * DH, j * QCH : (j + 1) * QCH],
                potB[0:DH, :],
                pb[:, QCH : 2 * QCH],
                op=MULT,
            )

        # --- A2A plumbing ---
        a2a_bufs = {}

        def emit_a2a_cin(p, j):
            if p not in a2a_bufs:
                cin = dram.tile([N_CORES * 128, QL], MM_DT, name=f"cin{p}")
                cout = dram.tile([N_CORES * 128, QL], MM_DT, name=f"cout{p}")
                a2a_bufs[p] = (cin, cout)
            cin = a2a_bufs[p][0]
            for s in (2 * j, 2 * j + 1):
                nc.sync.dma_start(
                    cin[s * 128 : (s + 1) * 128, :],
                    oft_own[p][:, s * QL : (s + 1) * QL],
                )

        def emit_a2a_trigger(p):
            cin, cout = a2a_bufs[p]
            nc.gpsimd.collective_compute(
                "AllToAll",
                mybir.AluOpType.bypass,
                replica_groups=[list(range(N_CORES))],
                ins=[cin[:]],
                outs=[cout[:]],
            )

        def emit_a2a_post(p):
            cin, cout = a2a_bufs[p]
            for rr in range(GROUPS):
                for bi in range(2):
                    src_rank = bi * GROUPS + rr
                    nc.sync.dma_start(
                        oft_all[2 * rr + p][:, bi * QL : (bi + 1) * QL],
                        cout[src_rank * 128 : (src_rank + 1) * 128, :],
                    )

        # ===== emission schedule =====
        # Long dummy burst: keeps the PE at full clock through the input-DMA
        # ramp (phase-A idle gaps >3.4us would re-throttle the HAM).
        warm_burst(44)
        emit_qk_phase(0)
        emit_qk_phase(1)
        for sb_i in range(NKB):
            emit_proj_v(sb_i)

        emit_attn_sweep(0, 3, 2)
        emit_wo_block(0)
        emit_wo_block(1)
        emit_attn_sweep(0, 1, 0)
        emit_a2a_trigger(0)
        emit_wo_block(2)
        emit_wo_block(3)

        emit_attn_sweep(1, 3, 2)
        emit_wo_block(4)
        emit_wo_block(5)
        emit_wo_block(6)
        emit_wo_block(7)
        nc.gpsimd.dma_start(bb_sb[:], bb_d[:])
        emit_attn_sweep(1, 1, 0)
        emit_a2a_post(0)
        emit_a2a_trigger(1)
        emit_a2a_post(1)

        # --- output projection on local 256-query slice of each batch ---
        # 8 PSUM slots (4 tiles x 2 column halves) held through both phases.
        out_tiles = [(0, 0), (0, 1), (1, 0), (1, 1)]  # (bi, qb)
        pys = [
            psum.tile([128, QCH], F32, tag=PTAGS[t], bufs=2, name=f"py{t}_{ech}")
            for t in range(4)
            for ech in range(2)
        ]

        # Gate matmuls: zero contribution (moving operand is the zeros tile),
        # but the stationary operand reads oft_own[1] cols 0:128 — written by
        # the LAST attention normalize (pair 1, chunk 0). Deliberate fence:
        # the PE queue is in-order, so without it the scheduler hoists these
        # out-proj accumulations (which wait on the collective's DMAs) into
        # the attention stream and stalls attention behind the A2A.
        for i in range(8):
            nc.tensor.matmul(
                pys[i][:], oft_own[1][:, 0:128], dmy[:], start=True, stop=False
            )

        def emit_out_phase(fs, last):
            for t, (bi, qb) in enumerate(out_tiles):
                for fi, f in enumerate(fs):
                    for ech in range(2):
                        nc.tensor.matmul(
                            pys[2 * t + ech][:],
                            oft_all[f][:, bi * QL + qb * 128 : bi * QL + (qb + 1) * 128],
                            wo_sb[:, f * D + ech * QCH : f * D + (ech + 1) * QCH],
                            start=False,
                            stop=(last and fi == len(fs) - 1),
                        )
                if last:  # per-tile drain: bias add + y DMA on rotating queues
                    ysb = work.tile([128, D], F32, tag="ysb", bufs=2, name=f"y{t}")
                    for ech in range(2):
                        nc.vector.tensor_tensor(
                            ysb[:, ech * QCH : (ech + 1) * QCH],
                            pys[2 * t + ech][:],
                            bb_sb[:, ech * QCH : (ech + 1) * QCH],
                            op=ADD,
                        )
                    row0 = bi * QL + qb * 128
                    for half in range(2):
                        qs[half % 2].dma_start(
                            y_d[row0 : row0 + 128, half * QCH : (half + 1) * QCH],
                            ysb[:, half * QCH : (half + 1) * QCH],
                        )

        emit_out_phase([0, 2, 4, 6], last=False)
        emit_out_phase([1, 3, 5, 7], last=True)


def build_program():
    nc = bacc.Bacc(
        "TRN2", target_bir_lowering=False, debug=False, num_devices=N_CORES
    )
    xT = nc.dram_tensor("xT", [D, S], BF16, kind="ExternalInput")
    wq = nc.dram_tensor("wq", [D, EH], BF16, kind="ExternalInput")
    wk = nc.dram_tensor("wk", [D, EH], BF16, kind="ExternalInput")
    wv = nc.dram_tensor("wv", [D, EH], BF16, kind="ExternalInput")
    wo = nc.dram_tensor("wo", [D, D], BF16, kind="ExternalInput")
    bb = nc.dram_tensor("bb", [128, D], F32, kind="ExternalInput")
    y = nc.dram_tensor("y", [2 * QL, D], F32, kind="ExternalOutput")
    with tile.TileContext(nc) as tc:
        _emit(nc, tc, xT.ap(), wq.ap(), wk.ap(), wv.ap(), wo.ap(), bb.ap(), y.ap())
    nc.compile()
    return nc


_cached_nc = None


def _get_nc():
    global _cached_nc
    if _cached_nc is None:
        _cached_nc = build_program()
    return _cached_nc


def make_in_maps(x, w_qkv, w_out, b_out):
    import ml_dtypes

    bf16 = ml_dtypes.bfloat16
    x = np.asarray(x, np.float32).astype(bf16)
    w_qkv = np.asarray(w_qkv, np.float32).astype(bf16)
    w_out = np.ascontiguousarray(np.asarray(w_out, np.float32).astype(bf16))
    b_out = np.asarray(b_out, np.float32)
    bb = np.ascontiguousarray(np.broadcast_to(b_out, (128, D)))
    in_maps = []
    for c in range(N_CORES):
        b, g = c // GROUPS, c % GROUPS
        in_maps.append(
            {
                "xT": np.ascontiguousarray(x[b].T),
                "wq": np.ascontiguousarray(w_qkv[:, g * EH : (g + 1) * EH]),
                "wk": np.ascontiguousarray(w_qkv[:, D + g * EH : D + (g + 1) * EH]),
                "wv": np.ascontiguousarray(
                    w_qkv[:, 2 * D + g * EH : 2 * D + (g + 1) * EH]
                ),
                "wo": w_out,
                "bb": bb,
            }
        )
    return in_maps


def assemble(results):
    # core c's y is [512, D]: rows [0,256) = batch 0 q-slice [256c, 256c+256),
    # rows [256,512) = batch 1 same slice.
    y = np.empty((B, S, D), np.float32)
    for c in range(N_CORES):
        yc = results[c]["y"]
        y[0, 256 * c : 256 * (c + 1), :] = yc[:256]
        y[1, 256 * c : 256 * (c + 1), :] = yc[256:]
    return y


def kernel(x, w_qkv, w_out, b_out, _trace=False, **run_kwargs):
    nc = _get_nc()
    in_maps = make_in_maps(x, w_qkv, w_out, b_out)
    res = run_bass_kernel_spmd(
        nc, in_maps, core_ids=list(range(N_CORES)), trace=_trace, **run_kwargs
    )
    out = assemble(res.results)
    if _trace:
        return out, res
    return out
